# revision 7
# baseline (speedup 1.0000x reference)
"""Adaptive-softmax cross-entropy loss on 8 Trainium2 NeuronCores.

Strategy (token-parallel with label-sorted routing):
  * Tokens are sorted by label on the host. Each core takes a contiguous
    chunk of 512 sorted tokens for the head softmax (vocab 16002), plus a
    contiguous chunk of the cluster-0 / cluster-1 token runs for the two
    tail softmaxes (vocab 12000 / 8000, with low-rank input projections).
  * Each core gathers its token rows from HBM with a transposing
    dma_gather (bf16), so activations land directly in [K,128]-tile layout
    for the TensorEngine.
  * Logits never need a max-subtraction here (|logit| ~ 4), so streaming
    softmax is: matmul chunk -> Exp on the Scalar engine with fused
    accum_out (per-token sum of exps), plus one fused DVE
    scalar_tensor_tensor per chunk that extracts the label logit via an
    iota==label one-hot multiply-reduce.
  * Device outputs per-token (sum_exp, label_logit) pairs; host finishes
    with log() and the masked mean (tiny O(tokens) work).
"""

import math

import numpy as np
import ml_dtypes

import concourse.bass as bass
import concourse.mybir as mybir
import concourse.tile as tile
from concourse import bacc
from concourse.bass_utils import run_bass_kernel_spmd
from concourse.masks import make_identity

CUTOFFS = (16000, 28000, 36000)
HID = 1024
NCORES = 8
CH = 1024  # vocab chunk width (2 PSUM banks)
BF16 = mybir.dt.bfloat16
F32 = mybir.dt.float32
NPBF16 = ml_dtypes.bfloat16
PAD_LABEL = -100000.0

# ---------------------------------------------------------------------------
# Workaround for this container's walrus build: CoreV3 codegen accepts only
# ONE embedded sync-wait per instruction, while Tile emits instructions whose
# sync_info carries one wait per producing logical processor. Legalize after
# scheduling: hoist all-but-one wait onto same-engine NoOps inserted directly
# before the instruction (same-engine program order makes this equivalent).
_nop_counter = [0]


def _legalize_sync_waits(nc, max_waits=1):
    for fn in nc.m.functions:
        for blk in fn.blocks:
            insts = blk.instructions
            if not any(
                inst.sync_info is not None
                and inst.sync_info.on_wait
                and len(inst.sync_info.on_wait) > max_waits
                for inst in insts
            ):
                continue
            new = []
            for inst in insts:
                si = inst.sync_info
                waits = list(si.on_wait) if (si is not None and si.on_wait) else []
                if len(waits) > max_waits:
                    for w in waits[:-max_waits]:
                        _nop_counter[0] += 1
                        nop = mybir.InstNoOp(
                            name=f"LW-{_nop_counter[0]}", ins=[], outs=[]
                        )
                        nop.engine = inst.engine
                        nop.sync_info = mybir.SyncInfo(on_wait=[w], on_update=[])
                        nc.register_instruction(nop, overwrite=True)
                        new.append(nop)
                    inst.sync_info = mybir.SyncInfo(
                        on_wait=waits[-max_waits:],
                        on_update=list(si.on_update) if si.on_update else [],
                    )
                new.append(inst)
            blk.instructions = new
# ---------------------------------------------------------------------------


def _cdiv(a, b):
    return (a + b - 1) // b


def _wrap_idxs(idxs, num):
    """dma_gather index layout: idx i lives at [i % 16, i // 16], and the
    16-partition block is replicated to all 8 gpsimd cores (128 partitions)."""
    assert num % 16 == 0 and len(idxs) == num
    a = np.asarray(idxs, np.int16).reshape(num // 16, 16).T  # [16, num/16]
    return np.tile(a, (8, 1))  # [128, num/16]


def build_graph(plan):
    """One SPMD graph, identical for all 8 cores."""
    ntok = plan["ntok"]
    tpc = plan["tpc"]  # head tokens per core (multiple of 128)
    cap0, cap1 = plan["cap0"], plan["cap1"]  # tail token capacity per core
    nbh, nb0, nb1 = tpc // 128, cap0 // 128, cap1 // 128
    nchh, nch0, nch1 = _cdiv(16002, CH), _cdiv(12000, CH), _cdiv(8000, CH)
    use_bias = plan["use_bias"]
    ncols = plan["ncols"]

    nc = bacc.Bacc(num_devices=NCORES)

    xt = nc.declare_dram_parameter("xt", [ntok, HID], BF16, isOutput=False)
    hw = nc.declare_dram_parameter("hw", [HID, 16002], BF16, isOutput=False)
    p0 = nc.declare_dram_parameter("p0", [HID, 256], BF16, isOutput=False)
    w0 = nc.declare_dram_parameter("w0", [256, 12000], BF16, isOutput=False)
    p1 = nc.declare_dram_parameter("p1", [HID, 64], BF16, isOutput=False)
    w1 = nc.declare_dram_parameter("w1", [64, 8000], BF16, isOutput=False)
    hidx = nc.declare_dram_parameter("hidx", [128, tpc // 16], mybir.dt.int16, isOutput=False)
    idx0 = nc.declare_dram_parameter("idx0", [128, cap0 // 16], mybir.dt.int16, isOutput=False)
    idx1 = nc.declare_dram_parameter("idx1", [128, cap1 // 16], mybir.dt.int16, isOutput=False)
    hsh = nc.declare_dram_parameter("hsh", [128, nbh, nchh], F32, isOutput=False)
    sh0 = nc.declare_dram_parameter("sh0", [128, nb0, nch0], F32, isOutput=False)
    sh1 = nc.declare_dram_parameter("sh1", [128, nb1, nch1], F32, isOutput=False)
    if use_bias:
        hb = nc.declare_dram_parameter("hb", [1, 16002], BF16, isOutput=False)
        b0 = nc.declare_dram_parameter("b0", [1, 12000], BF16, isOutput=False)
        b1 = nc.declare_dram_parameter("b1", [1, 8000], BF16, isOutput=False)
        pb0 = nc.declare_dram_parameter("pb0", [1, 256], BF16, isOutput=False)
        pb1 = nc.declare_dram_parameter("pb1", [1, 64], BF16, isOutput=False)
    out = nc.declare_dram_parameter("out", [128, ncols], F32, isOutput=True)

    Exp = mybir.ActivationFunctionType.Exp

    with tile.TileContext(nc) as tc:
        from contextlib import ExitStack

        with ExitStack() as ctx:
            const = ctx.enter_context(tc.tile_pool(name="const", bufs=1))
            wpool = ctx.enter_context(tc.tile_pool(name="w", bufs=2))
            spool = ctx.enter_context(tc.tile_pool(name="scratch", bufs=3))

            # --- constants / indices / gathered activations ---
            hidx_sb = const.tile([128, tpc // 16], mybir.dt.int16)
            nc.sync.dma_start(out=hidx_sb[:, :], in_=hidx[:, :])
            idx0_sb = const.tile([128, cap0 // 16], mybir.dt.int16)
            nc.sync.dma_start(out=idx0_sb[:, :], in_=idx0[:, :])
            idx1_sb = const.tile([128, cap1 // 16], mybir.dt.int16)
            nc.sync.dma_start(out=idx1_sb[:, :], in_=idx1[:, :])

            hsh_sb = const.tile([128, nbh, nchh], F32)
            nc.sync.dma_start(out=hsh_sb[:, :, :], in_=hsh[:, :, :])
            sh0_sb = const.tile([128, nb0, nch0], F32)
            nc.sync.dma_start(out=sh0_sb[:, :, :], in_=sh0[:, :, :])
            sh1_sb = const.tile([128, nb1, nch1], F32)
            nc.sync.dma_start(out=sh1_sb[:, :, :], in_=sh1[:, :, :])

            xh = const.tile([128, 8, tpc], BF16)
            nc.gpsimd.dma_gather(
                xh[:, :, :], xt[:, :], hidx_sb[:, :],
                num_idxs=tpc, num_idxs_reg=tpc, elem_size=HID, transpose=True,
            )
            x0 = const.tile([128, 8, cap0], BF16)
            nc.gpsimd.dma_gather(
                x0[:, :, :], xt[:, :], idx0_sb[:, :],
                num_idxs=cap0, num_idxs_reg=cap0, elem_size=HID, transpose=True,
            )
            x1 = const.tile([128, 8, cap1], BF16)
            nc.gpsimd.dma_gather(
                x1[:, :, :], xt[:, :], idx1_sb[:, :],
                num_idxs=cap1, num_idxs_reg=cap1, elem_size=HID, transpose=True,
            )

            iota_i = const.tile([128, CH], mybir.dt.int32)
            nc.gpsimd.iota(iota_i[:, :], pattern=[[1, CH]], base=0, channel_multiplier=0)
            iota_f = const.tile([128, CH], F32)
            nc.vector.tensor_copy(iota_f[:, :], iota_i[:, :])

            identity = const.tile([128, 128], BF16)
            make_identity(nc, identity[:, :])

            ones1 = const.tile([1, 128], BF16)
            nc.vector.memset(ones1[:, :], 1.0)

            bias_sb = {}
            if use_bias:
                for name, ap, n in (
                    ("hb", hb, 16002), ("b0", b0, 12000), ("b1", b1, 8000),
                    ("pb0", pb0, 256), ("pb1", pb1, 64),
                ):
                    t = const.tile([1, n], BF16, tag=f"bias_{name}")
                    nc.sync.dma_start(out=t[:, :], in_=ap[:, :])
                    bias_sb[name] = t

            # per-block/chunk partial results
            hse = const.tile([128, nbh, nchh], F32)
            hll = const.tile([128, nbh, nchh], F32)
            se0 = const.tile([128, nb0, nch0], F32)
            ll0 = const.tile([128, nb0, nch0], F32)
            se1 = const.tile([128, nb1, nch1], F32)
            ll1 = const.tile([128, nb1, nch1], F32)
            out_sb = const.tile([128, ncols], F32)

            # load tail projection weights
            p0_sb = const.tile([128, 8, 256], BF16)
            nc.sync.dma_start(out=p0_sb[:, :, :], in_=p0.ap().rearrange("(c p) n -> p c n", p=128))
            p1_sb = const.tile([128, 8, 64], BF16)
            nc.sync.dma_start(out=p1_sb[:, :, :], in_=p1.ap().rearrange("(c p) n -> p c n", p=128))

            # --- tail projections: h = x @ pW (+pb), then transpose to [proj, tok] ---
            hT0 = const.tile([128, 2, cap0], BF16)
            hT1 = const.tile([64, 1, cap1], BF16)

            with tc.tile_pool(name="ppsum", bufs=2, space="PSUM") as ppsum:
                for tb in range(nb0):
                    ph = ppsum.tile([128, 256], F32, tag="proj")
                    for c in range(8):
                        nc.tensor.matmul(
                            ph[:, :], x0[:, c, bass.ts(tb, 128)], p0_sb[:, c, :],
                            start=(c == 0), stop=(c == 7 and not use_bias),
                        )
                    if use_bias:
                        nc.tensor.matmul(
                            ph[:, :], ones1[0:1, :], bias_sb["pb0"][0:1, :],
                            start=False, stop=True,
                        )
                    hsb = spool.tile([128, 256], BF16, tag="hsb")
                    nc.scalar.copy(hsb[:, :], ph[:, :])
                    for j in range(2):
                        pt = ppsum.tile([128, 128], BF16, tag="ptr")
                        nc.tensor.transpose(pt[:, :], hsb[:, bass.ts(j, 128)], identity[:, :])
                        nc.scalar.copy(hT0[:, j, bass.ts(tb, 128)], pt[:, :])
                for tb in range(nb1):
                    ph = ppsum.tile([128, 64], F32, tag="proj")
                    for c in range(8):
                        nc.tensor.matmul(
                            ph[:, :], x1[:, c, bass.ts(tb, 128)], p1_sb[:, c, :],
                            start=(c == 0), stop=(c == 7 and not use_bias),
                        )
                    if use_bias:
                        nc.tensor.matmul(
                            ph[:, :], ones1[0:1, :], bias_sb["pb1"][0:1, :],
                            start=False, stop=True,
                        )
                    hsb = spool.tile([128, 64], BF16, tag="hsb")
                    nc.scalar.copy(hsb[:, :], ph[:, :])
                    pt = ppsum.tile([64, 128], BF16, tag="ptr")
                    nc.tensor.transpose(pt[:, :], hsb[:, :], identity[:, :])
                    nc.scalar.copy(hT1[:, 0, bass.ts(tb, 128)], pt[:, :])

            # --- streaming softmax over vocab chunks ---
            def stream(xT, KT, kpart, W_ap, V, nb, labsh, se_sb, ll_sb, bias):
                nch = _cdiv(V, CH)
                Wr = W_ap.ap().rearrange("(c p) n -> p c n", p=kpart)
                for ch in range(nch):
                    v0 = ch * CH
                    vw = min(CH, V - v0)
                    wt = wpool.tile([kpart, KT, vw], BF16, tag="wt")
                    nc.sync.dma_start(out=wt[:, :, :], in_=Wr[:, :, v0 : v0 + vw])
                    for tb in range(nb):
                        ps = lpsum.tile([128, CH], F32, tag="logits")
                        for c in range(KT):
                            for s0 in range(0, vw, 512):
                                sw = min(512, vw - s0)
                                nc.tensor.matmul(
                                    ps[:, s0 : s0 + sw],
                                    xT[:, c, bass.ts(tb, 128)],
                                    wt[:, c, s0 : s0 + sw],
                                    start=(c == 0),
                                    stop=(c == KT - 1 and bias is None),
                                )
                        if bias is not None:
                            for s0 in range(0, vw, 512):
                                sw = min(512, vw - s0)
                                nc.tensor.matmul(
                                    ps[:, s0 : s0 + sw],
                                    ones1[0:1, :],
                                    bias[0:1, v0 + s0 : v0 + s0 + sw],
                                    start=False, stop=True,
                                )
                        ex = spool.tile([128, CH], F32, tag="ex")
                        nc.scalar.activation(
                            ex[:, :vw], ps[:, :vw], Exp,
                            accum_out=se_sb[:, tb, ch : ch + 1],
                        )
                        st = spool.tile([128, CH], F32, tag="st")
                        nc.vector.scalar_tensor_tensor(
                            out=st[:, :vw],
                            in0=iota_f[:, :vw],
                            scalar=labsh[:, tb, ch : ch + 1],
                            in1=ps[:, :vw],
                            op0=mybir.AluOpType.is_equal,
                            op1=mybir.AluOpType.mult,
                            accum_out=ll_sb[:, tb, ch : ch + 1],
                        )

            with tc.tile_pool(name="lpsum", bufs=4, space="PSUM") as lpsum:
                stream(xh, 8, 128, hw, 16002, nbh, hsh_sb, hse, hll,
                       bias_sb.get("hb"))
                stream(hT0, 2, 128, w0, 12000, nb0, sh0_sb, se0, ll0,
                       bias_sb.get("b0"))
                stream(hT1, 1, 64, w1, 8000, nb1, sh1_sb, se1, ll1,
                       bias_sb.get("b1"))

                # --- reduce per-chunk partials into output columns ---
                col = 0
                for se_sb, ll_sb, nb, nch in (
                    (hse, hll, nbh, nchh),
                    (se0, ll0, nb0, nch0),
                    (se1, ll1, nb1, nch1),
                ):
                    for tb in range(nb):
                        nc.vector.reduce_sum(
                            out=out_sb[:, col : col + 1],
                            in_=se_sb[:, tb, :], axis=mybir.AxisListType.X,
                        )
                        nc.vector.reduce_sum(
                            out=out_sb[:, col + nb : col + nb + 1],
                            in_=ll_sb[:, tb, :], axis=mybir.AxisListType.X,
                        )
                        col += 1
                    col += nb

                nc.sync.dma_start(out=out[:, :], in_=out_sb[:, :])

    nc.compile()
    _legalize_sync_waits(nc)
    return nc


def make_plan_and_maps(inp, labels, head_W, head_b, t0_pW, t0_pb, t0_W, t0_b,
                       t1_pW, t1_pb, t1_W, t1_b):
    X = np.ascontiguousarray(np.asarray(inp, np.float32).reshape(-1, HID))
    labels = np.asarray(labels).astype(np.int64).reshape(-1)
    ntok = X.shape[0]
    assert ntok % (NCORES * 128) == 0, ntok

    order = np.argsort(labels, kind="stable")
    slab = labels[order]
    head_labels = labels.copy()
    m0 = (labels >= CUTOFFS[0]) & (labels < CUTOFFS[1])
    m1 = (labels >= CUTOFFS[1]) & (labels < CUTOFFS[2])
    head_labels[m0] = CUTOFFS[0]
    head_labels[m1] = CUTOFFS[0] + 1

    tpc = ntok // NCORES
    head_tok = [order[c * tpc : (c + 1) * tpc] for c in range(NCORES)]

    def split_cluster(lo, hi):
        toks = order[(slab >= lo) & (slab < hi)]  # sorted by label
        n = len(toks)
        per = _cdiv(max(n, 1), NCORES)
        cap = max(_cdiv(per, 128) * 128, 128)
        chunks, valid = [], []
        for c in range(NCORES):
            chunk = toks[c * per : (c + 1) * per]
            v = len(chunk)
            pad = np.zeros(cap - v, np.int64)
            chunks.append(np.concatenate([chunk, pad]))
            valid.append(v)
        return chunks, valid, cap

    c0_tok, c0_valid, cap0 = split_cluster(CUTOFFS[0], CUTOFFS[1])
    c1_tok, c1_valid, cap1 = split_cluster(CUTOFFS[1], CUTOFFS[2])

    nbh, nb0, nb1 = tpc // 128, cap0 // 128, cap1 // 128
    nchh, nch0, nch1 = _cdiv(16002, CH), _cdiv(12000, CH), _cdiv(8000, CH)
    ncols = 2 * (nbh + nb0 + nb1)

    use_bias = any(
        float(np.abs(np.asarray(b, np.float32)).max()) > 0
        for b in (head_b, t0_b, t1_b, t0_pb, t1_pb)
    )

    plan = dict(ntok=ntok, tpc=tpc, cap0=cap0, cap1=cap1, ncols=ncols,
                use_bias=use_bias, head_tok=head_tok, c0_tok=c0_tok,
                c1_tok=c1_tok, c0_valid=c0_valid, c1_valid=c1_valid,
                labels=labels, head_labels=head_labels)

    def labshift(tok_list, lab_vals, nb, nch, valid):
        """[128, nb, nch] f32: label - chunk_base, PAD_LABEL on padding."""
        a = np.full((nb * 128,), PAD_LABEL, np.float32)
        a[:valid] = lab_vals[tok_list[:valid]].astype(np.float32)
        a = a.reshape(nb, 128).T  # [128, nb]
        base = (np.arange(nch, dtype=np.float32) * CH)[None, None, :]
        return np.ascontiguousarray(a[:, :, None] - base)

    Xb = X.astype(NPBF16)
    shared = {
        "xt": Xb,
        "hw": np.asarray(head_W, np.float32).astype(NPBF16),
        "p0": np.asarray(t0_pW, np.float32).astype(NPBF16),
        "w0": np.asarray(t0_W, np.float32).astype(NPBF16),
        "p1": np.asarray(t1_pW, np.float32).astype(NPBF16),
        "w1": np.asarray(t1_W, np.float32).astype(NPBF16),
    }
    if use_bias:
        shared["hb"] = np.asarray(head_b, np.float32).astype(NPBF16)[None, :]
        shared["b0"] = np.asarray(t0_b, np.float32).astype(NPBF16)[None, :]
        shared["b1"] = np.asarray(t1_b, np.float32).astype(NPBF16)[None, :]
        shared["pb0"] = np.asarray(t0_pb, np.float32).astype(NPBF16)[None, :]
        shared["pb1"] = np.asarray(t1_pb, np.float32).astype(NPBF16)[None, :]

    in_maps = []
    for c in range(NCORES):
        m = dict(shared)
        m["hidx"] = _wrap_idxs(head_tok[c], tpc)
        m["idx0"] = _wrap_idxs(c0_tok[c], cap0)
        m["idx1"] = _wrap_idxs(c1_tok[c], cap1)
        m["hsh"] = labshift(head_tok[c], head_labels, nbh, nchh, tpc)
        m["sh0"] = labshift(c0_tok[c], labels - CUTOFFS[0], nb0, nch0, c0_valid[c])
        m["sh1"] = labshift(c1_tok[c], labels - CUTOFFS[1], nb1, nch1, c1_valid[c])
        in_maps.append(m)
    return plan, in_maps


def assemble_loss(plan, outs):
    """outs: list of per-core [128, ncols] f32 arrays -> mean loss (f64)."""
    ntok = plan["ntok"]
    labels = plan["labels"]
    tpc = plan["tpc"]
    nbh = tpc // 128
    nb0 = plan["cap0"] // 128
    nb1 = plan["cap1"] // 128
    total = 0.0
    for c in range(NCORES):
        o = np.asarray(outs[c], np.float64)
        col = 0
        for tok_list, nb, valid in (
            (plan["head_tok"][c], nbh, tpc),
            (plan["c0_tok"][c], nb0, plan["c0_valid"][c]),
            (plan["c1_tok"][c], nb1, plan["c1_valid"][c]),
        ):
            se = o[:, col : col + nb].T.reshape(-1)[:valid]
            ll = o[:, col + nb : col + 2 * nb].T.reshape(-1)[:valid]
            w = (labels[tok_list[:valid]] != 0).astype(np.float64)
            total += float(np.dot(w, np.log(se) - ll))
            col += 2 * nb
    return total / ntok


_CACHE = {}


def kernel(inp, labels, head_W, head_b, t0_pW, t0_pb, t0_W, t0_b,
           t1_pW, t1_pb, t1_W, t1_b):
    plan, in_maps = make_plan_and_maps(
        inp, labels, head_W, head_b, t0_pW, t0_pb, t0_W, t0_b,
        t1_pW, t1_pb, t1_W, t1_b)
    key = (plan["ntok"], plan["tpc"], plan["cap0"], plan["cap1"], plan["use_bias"])
    if key not in _CACHE:
        _CACHE[key] = build_graph(plan)
    nc = _CACHE[key]
    res = run_bass_kernel_spmd(nc, in_maps, core_ids=list(range(NCORES)))
    outs = [res.results[c]["out"] for c in range(NCORES)]
    loss = assemble_loss(plan, outs)
    return np.asarray(loss, dtype=np.float32)


# revision 9
# speedup vs baseline: 25.0552x; 25.0552x over previous
"""Adaptive-softmax cross-entropy loss on 8 Trainium2 NeuronCores.

Strategy (token-parallel with label-sorted routing):
  * Tokens are sorted by label on the host. Each core takes a contiguous
    chunk of 512 sorted tokens for the head softmax (vocab 16002), plus a
    contiguous chunk of the cluster-0 / cluster-1 token runs for the two
    tail softmaxes (vocab 12000 / 8000, with low-rank input projections).
  * Each core gathers its token rows from HBM with a transposing
    dma_gather (bf16), so activations land directly in [K,128]-tile layout
    for the TensorEngine.
  * Logits never need a max-subtraction here (|logit| ~ 4), so streaming
    softmax is: matmul chunk -> Exp on the Scalar engine with fused
    accum_out (per-token sum of exps), plus one fused DVE
    scalar_tensor_tensor per chunk that extracts the label logit via an
    iota==label one-hot multiply-reduce.
  * Device outputs per-token (sum_exp, label_logit) pairs; host finishes
    with log() and the masked mean (tiny O(tokens) work).
"""

import math
from contextlib import ExitStack

import numpy as np
import ml_dtypes

import concourse.bass as bass
import concourse.mybir as mybir
import concourse.tile as tile
from concourse import bacc
from concourse.bass_utils import run_bass_kernel_spmd
from concourse.masks import make_identity

CUTOFFS = (16000, 28000, 36000)
HID = 1024
NCORES = 8
CH = 1024  # vocab chunk width (2 PSUM banks)
BF16 = mybir.dt.bfloat16
F32 = mybir.dt.float32
NPBF16 = ml_dtypes.bfloat16
PAD_LABEL = -100000.0

# ---------------------------------------------------------------------------
# Workaround for this container's walrus build: CoreV3 codegen accepts only
# ONE embedded sync-wait per instruction, while Tile emits instructions whose
# sync_info carries one wait per producing logical processor. Legalize after
# scheduling: hoist all-but-one wait onto same-engine NoOps inserted directly
# before the instruction (same-engine program order makes this equivalent).
_nop_counter = [0]


def _legalize_sync_waits(nc, max_waits=1):
    for fn in nc.m.functions:
        for blk in fn.blocks:
            insts = blk.instructions
            if not any(
                inst.sync_info is not None
                and inst.sync_info.on_wait
                and len(inst.sync_info.on_wait) > max_waits
                for inst in insts
            ):
                continue
            new = []
            for inst in insts:
                si = inst.sync_info
                waits = list(si.on_wait) if (si is not None and si.on_wait) else []
                if len(waits) > max_waits:
                    for w in waits[:-max_waits]:
                        _nop_counter[0] += 1
                        nop = mybir.InstNoOp(
                            name=f"LW-{_nop_counter[0]}", ins=[], outs=[]
                        )
                        nop.engine = inst.engine
                        nop.sync_info = mybir.SyncInfo(on_wait=[w], on_update=[])
                        nc.register_instruction(nop, overwrite=True)
                        new.append(nop)
                    inst.sync_info = mybir.SyncInfo(
                        on_wait=waits[-max_waits:],
                        on_update=list(si.on_update) if si.on_update else [],
                    )
                new.append(inst)
            blk.instructions = new
# ---------------------------------------------------------------------------


def _cdiv(a, b):
    return (a + b - 1) // b


def _wrap_idxs(idxs, num):
    """dma_gather index layout: idx i lives at [i % 16, i // 16], and the
    16-partition block is replicated to all 8 gpsimd cores (128 partitions)."""
    assert num % 16 == 0 and len(idxs) == num
    a = np.asarray(idxs, np.int16).reshape(num // 16, 16).T  # [16, num/16]
    return np.tile(a, (8, 1))  # [128, num/16]


def build_graph(plan, reps=1):
    """One SPMD graph, identical for all 8 cores.

    reps > 1 unrolls the whole kernel body back-to-back inside the NEFF so
    a timing harness can measure marginal (steady-state) per-rep cost,
    cancelling host/dispatch overhead: t = (T(K) - T(1)) / (K - 1)."""
    ntok = plan["ntok"]
    tpc = plan["tpc"]  # head tokens per core (multiple of 128)
    cap0, cap1 = plan["cap0"], plan["cap1"]  # tail token capacity per core
    nbh, nb0, nb1 = tpc // 128, cap0 // 128, cap1 // 128
    nchh, nch0, nch1 = _cdiv(16002, CH), _cdiv(12000, CH), _cdiv(8000, CH)
    use_bias = plan["use_bias"]
    ncols = plan["ncols"]

    nc = bacc.Bacc(num_devices=NCORES)

    xt = nc.declare_dram_parameter("xt", [ntok, HID], BF16, isOutput=False)
    hw = nc.declare_dram_parameter("hw", [HID, 16002], BF16, isOutput=False)
    p0 = nc.declare_dram_parameter("p0", [HID, 256], BF16, isOutput=False)
    w0 = nc.declare_dram_parameter("w0", [256, 12000], BF16, isOutput=False)
    p1 = nc.declare_dram_parameter("p1", [HID, 64], BF16, isOutput=False)
    w1 = nc.declare_dram_parameter("w1", [64, 8000], BF16, isOutput=False)
    hidx = nc.declare_dram_parameter("hidx", [128, tpc // 16], mybir.dt.int16, isOutput=False)
    idx0 = nc.declare_dram_parameter("idx0", [128, cap0 // 16], mybir.dt.int16, isOutput=False)
    idx1 = nc.declare_dram_parameter("idx1", [128, cap1 // 16], mybir.dt.int16, isOutput=False)
    hsh = nc.declare_dram_parameter("hsh", [128, nbh, nchh], F32, isOutput=False)
    sh0 = nc.declare_dram_parameter("sh0", [128, nb0, nch0], F32, isOutput=False)
    sh1 = nc.declare_dram_parameter("sh1", [128, nb1, nch1], F32, isOutput=False)
    if use_bias:
        hb = nc.declare_dram_parameter("hb", [1, 16002], BF16, isOutput=False)
        b0 = nc.declare_dram_parameter("b0", [1, 12000], BF16, isOutput=False)
        b1 = nc.declare_dram_parameter("b1", [1, 8000], BF16, isOutput=False)
        pb0 = nc.declare_dram_parameter("pb0", [1, 256], BF16, isOutput=False)
        pb1 = nc.declare_dram_parameter("pb1", [1, 64], BF16, isOutput=False)
    out = nc.declare_dram_parameter("out", [128, ncols], F32, isOutput=True)

    Exp = mybir.ActivationFunctionType.Exp

    with tile.TileContext(nc) as tc:
        with ExitStack() as ctx:
            const = ctx.enter_context(tc.tile_pool(name="const", bufs=1))
            wpool = ctx.enter_context(tc.tile_pool(name="w", bufs=2))
            spool = ctx.enter_context(tc.tile_pool(name="scratch", bufs=3))

            # --- setup: indices / labels / constants (outside rep loop) ---
            hidx_sb = const.tile([128, tpc // 16], mybir.dt.int16)
            nc.sync.dma_start(out=hidx_sb[:, :], in_=hidx[:, :])
            idx0_sb = const.tile([128, cap0 // 16], mybir.dt.int16)
            nc.sync.dma_start(out=idx0_sb[:, :], in_=idx0[:, :])
            idx1_sb = const.tile([128, cap1 // 16], mybir.dt.int16)
            nc.sync.dma_start(out=idx1_sb[:, :], in_=idx1[:, :])

            hsh_sb = const.tile([128, nbh, nchh], F32)
            nc.sync.dma_start(out=hsh_sb[:, :, :], in_=hsh[:, :, :])
            sh0_sb = const.tile([128, nb0, nch0], F32)
            nc.sync.dma_start(out=sh0_sb[:, :, :], in_=sh0[:, :, :])
            sh1_sb = const.tile([128, nb1, nch1], F32)
            nc.sync.dma_start(out=sh1_sb[:, :, :], in_=sh1[:, :, :])

            iota_i = const.tile([128, CH], mybir.dt.int32)
            nc.gpsimd.iota(iota_i[:, :], pattern=[[1, CH]], base=0, channel_multiplier=0)
            iota_f = const.tile([128, CH], F32)
            nc.vector.tensor_copy(iota_f[:, :], iota_i[:, :])

            identity = const.tile([128, 128], BF16)
            make_identity(nc, identity[:, :])

            ones1 = const.tile([1, 128], BF16)
            nc.vector.memset(ones1[:, :], 1.0)

            bias_sb = {}
            if use_bias:
                for name, ap, n in (
                    ("hb", hb, 16002), ("b0", b0, 12000), ("b1", b1, 8000),
                    ("pb0", pb0, 256), ("pb1", pb1, 64),
                ):
                    t = const.tile([1, n], BF16, tag=f"bias_{name}")
                    nc.sync.dma_start(out=t[:, :], in_=ap[:, :])
                    bias_sb[name] = t

            p0_sb = const.tile([128, 8, 256], BF16)
            nc.sync.dma_start(out=p0_sb[:, :, :], in_=p0.ap().rearrange("(c p) n -> p c n", p=128))
            p1_sb = const.tile([128, 8, 64], BF16)
            nc.sync.dma_start(out=p1_sb[:, :, :], in_=p1.ap().rearrange("(c p) n -> p c n", p=128))

            # persistent result tiles (rewritten each rep)
            xh = const.tile([128, 8, tpc], BF16)
            x0 = const.tile([128, 8, cap0], BF16)
            x1 = const.tile([128, 8, cap1], BF16)
            hT0 = const.tile([128, 2, cap0], BF16)
            hT1 = const.tile([64, 1, cap1], BF16)
            hse = const.tile([128, nbh, nchh], F32)
            hll = const.tile([128, nbh, nchh], F32)
            se0 = const.tile([128, nb0, nch0], F32)
            ll0 = const.tile([128, nb0, nch0], F32)
            se1 = const.tile([128, nb1, nch1], F32)
            ll1 = const.tile([128, nb1, nch1], F32)
            out_sb = const.tile([128, ncols], F32)

            def emit_body():
                # gather this core's token rows (transposed, bf16)
                nc.gpsimd.dma_gather(
                    xh[:, :, :], xt[:, :], hidx_sb[:, :],
                    num_idxs=tpc, num_idxs_reg=tpc, elem_size=HID, transpose=True,
                )
                nc.gpsimd.dma_gather(
                    x0[:, :, :], xt[:, :], idx0_sb[:, :],
                    num_idxs=cap0, num_idxs_reg=cap0, elem_size=HID, transpose=True,
                )
                nc.gpsimd.dma_gather(
                    x1[:, :, :], xt[:, :], idx1_sb[:, :],
                    num_idxs=cap1, num_idxs_reg=cap1, elem_size=HID, transpose=True,
                )

                # --- tail projections: h = x @ pW (+pb) -> transpose [proj, tok] ---
                with tc.tile_pool(name="ppsum", bufs=2, space="PSUM") as ppsum:
                    for tb in range(nb0):
                        ph = ppsum.tile([128, 256], F32, tag="proj")
                        for c in range(8):
                            nc.tensor.matmul(
                                ph[:, :], x0[:, c, bass.ts(tb, 128)], p0_sb[:, c, :],
                                start=(c == 0), stop=(c == 7 and not use_bias),
                            )
                        if use_bias:
                            nc.tensor.matmul(
                                ph[:, :], ones1[0:1, :], bias_sb["pb0"][0:1, :],
                                start=False, stop=True,
                            )
                        hsb = spool.tile([128, 256], BF16, tag="hsb")
                        nc.scalar.copy(hsb[:, :], ph[:, :])
                        for j in range(2):
                            pt = ppsum.tile([128, 128], BF16, tag="ptr")
                            nc.tensor.transpose(pt[:, :], hsb[:, bass.ts(j, 128)], identity[:, :])
                            nc.scalar.copy(hT0[:, j, bass.ts(tb, 128)], pt[:, :])
                    for tb in range(nb1):
                        ph = ppsum.tile([128, 64], F32, tag="proj")
                        for c in range(8):
                            nc.tensor.matmul(
                                ph[:, :], x1[:, c, bass.ts(tb, 128)], p1_sb[:, c, :],
                                start=(c == 0), stop=(c == 7 and not use_bias),
                            )
                        if use_bias:
                            nc.tensor.matmul(
                                ph[:, :], ones1[0:1, :], bias_sb["pb1"][0:1, :],
                                start=False, stop=True,
                            )
                        hsb = spool.tile([128, 64], BF16, tag="hsb")
                        nc.scalar.copy(hsb[:, :], ph[:, :])
                        pt = ppsum.tile([64, 128], BF16, tag="ptr")
                        nc.tensor.transpose(pt[:, :], hsb[:, :], identity[:, :])
                        nc.scalar.copy(hT1[:, 0, bass.ts(tb, 128)], pt[:, :])

                # --- streaming softmax over vocab chunks ---
                def stream(xT, KT, kpart, W_ap, V, nb, labsh, se_sb, ll_sb, bias):
                    nch = _cdiv(V, CH)
                    Wr = W_ap.ap().rearrange("(c p) n -> p c n", p=kpart)
                    for ch in range(nch):
                        v0 = ch * CH
                        vw = min(CH, V - v0)
                        wt = wpool.tile([kpart, KT, vw], BF16, tag="wt")
                        nc.sync.dma_start(out=wt[:, :, :], in_=Wr[:, :, v0 : v0 + vw])
                        for tb in range(nb):
                            ps = lpsum.tile([128, CH], F32, tag="logits")
                            for c in range(KT):
                                for s0 in range(0, vw, 512):
                                    sw = min(512, vw - s0)
                                    nc.tensor.matmul(
                                        ps[:, s0 : s0 + sw],
                                        xT[:, c, bass.ts(tb, 128)],
                                        wt[:, c, s0 : s0 + sw],
                                        start=(c == 0),
                                        stop=(c == KT - 1 and bias is None),
                                    )
                            if bias is not None:
                                for s0 in range(0, vw, 512):
                                    sw = min(512, vw - s0)
                                    nc.tensor.matmul(
                                        ps[:, s0 : s0 + sw],
                                        ones1[0:1, :],
                                        bias[0:1, v0 + s0 : v0 + s0 + sw],
                                        start=False, stop=True,
                                    )
                            ex = spool.tile([128, CH], F32, tag="ex")
                            nc.scalar.activation(
                                ex[:, :vw], ps[:, :vw], Exp,
                                accum_out=se_sb[:, tb, ch : ch + 1],
                            )
                            st = spool.tile([128, CH], F32, tag="st")
                            nc.vector.scalar_tensor_tensor(
                                out=st[:, :vw],
                                in0=iota_f[:, :vw],
                                scalar=labsh[:, tb, ch : ch + 1],
                                in1=ps[:, :vw],
                                op0=mybir.AluOpType.is_equal,
                                op1=mybir.AluOpType.mult,
                                accum_out=ll_sb[:, tb, ch : ch + 1],
                            )

                with tc.tile_pool(name="lpsum", bufs=4, space="PSUM") as lpsum:
                    stream(xh, 8, 128, hw, 16002, nbh, hsh_sb, hse, hll,
                           bias_sb.get("hb"))
                    stream(hT0, 2, 128, w0, 12000, nb0, sh0_sb, se0, ll0,
                           bias_sb.get("b0"))
                    stream(hT1, 1, 64, w1, 8000, nb1, sh1_sb, se1, ll1,
                           bias_sb.get("b1"))

                    # --- reduce per-chunk partials into output columns ---
                    col = 0
                    for se_sb, ll_sb, nb, nch in (
                        (hse, hll, nbh, nchh),
                        (se0, ll0, nb0, nch0),
                        (se1, ll1, nb1, nch1),
                    ):
                        for tb in range(nb):
                            nc.vector.reduce_sum(
                                out=out_sb[:, col : col + 1],
                                in_=se_sb[:, tb, :], axis=mybir.AxisListType.X,
                            )
                            nc.vector.reduce_sum(
                                out=out_sb[:, col + nb : col + nb + 1],
                                in_=ll_sb[:, tb, :], axis=mybir.AxisListType.X,
                            )
                            col += 1
                        col += nb

                    nc.sync.dma_start(out=out[:, :], in_=out_sb[:, :])

            for _rep in range(reps):
                emit_body()

    nc.compile()
    _legalize_sync_waits(nc)
    return nc


def make_plan_and_maps(inp, labels, head_W, head_b, t0_pW, t0_pb, t0_W, t0_b,
                       t1_pW, t1_pb, t1_W, t1_b):
    X = np.ascontiguousarray(np.asarray(inp, np.float32).reshape(-1, HID))
    labels = np.asarray(labels).astype(np.int64).reshape(-1)
    ntok = X.shape[0]
    assert ntok % (NCORES * 128) == 0, ntok

    order = np.argsort(labels, kind="stable")
    slab = labels[order]
    head_labels = labels.copy()
    m0 = (labels >= CUTOFFS[0]) & (labels < CUTOFFS[1])
    m1 = (labels >= CUTOFFS[1]) & (labels < CUTOFFS[2])
    head_labels[m0] = CUTOFFS[0]
    head_labels[m1] = CUTOFFS[0] + 1

    tpc = ntok // NCORES
    head_tok = [order[c * tpc : (c + 1) * tpc] for c in range(NCORES)]

    def split_cluster(lo, hi):
        toks = order[(slab >= lo) & (slab < hi)]  # sorted by label
        n = len(toks)
        per = _cdiv(max(n, 1), NCORES)
        cap = max(_cdiv(per, 128) * 128, 128)
        chunks, valid = [], []
        for c in range(NCORES):
            chunk = toks[c * per : (c + 1) * per]
            v = len(chunk)
            pad = np.zeros(cap - v, np.int64)
            chunks.append(np.concatenate([chunk, pad]))
            valid.append(v)
        return chunks, valid, cap

    c0_tok, c0_valid, cap0 = split_cluster(CUTOFFS[0], CUTOFFS[1])
    c1_tok, c1_valid, cap1 = split_cluster(CUTOFFS[1], CUTOFFS[2])

    nbh, nb0, nb1 = tpc // 128, cap0 // 128, cap1 // 128
    nchh, nch0, nch1 = _cdiv(16002, CH), _cdiv(12000, CH), _cdiv(8000, CH)
    ncols = 2 * (nbh + nb0 + nb1)

    use_bias = any(
        float(np.abs(np.asarray(b, np.float32)).max()) > 0
        for b in (head_b, t0_b, t1_b, t0_pb, t1_pb)
    )

    plan = dict(ntok=ntok, tpc=tpc, cap0=cap0, cap1=cap1, ncols=ncols,
                use_bias=use_bias, head_tok=head_tok, c0_tok=c0_tok,
                c1_tok=c1_tok, c0_valid=c0_valid, c1_valid=c1_valid,
                labels=labels, head_labels=head_labels)

    def labshift(tok_list, lab_vals, nb, nch, valid):
        """[128, nb, nch] f32: label - chunk_base, PAD_LABEL on padding."""
        a = np.full((nb * 128,), PAD_LABEL, np.float32)
        a[:valid] = lab_vals[tok_list[:valid]].astype(np.float32)
        a = a.reshape(nb, 128).T  # [128, nb]
        base = (np.arange(nch, dtype=np.float32) * CH)[None, None, :]
        return np.ascontiguousarray(a[:, :, None] - base)

    Xb = X.astype(NPBF16)
    shared = {
        "xt": Xb,
        "hw": np.asarray(head_W, np.float32).astype(NPBF16),
        "p0": np.asarray(t0_pW, np.float32).astype(NPBF16),
        "w0": np.asarray(t0_W, np.float32).astype(NPBF16),
        "p1": np.asarray(t1_pW, np.float32).astype(NPBF16),
        "w1": np.asarray(t1_W, np.float32).astype(NPBF16),
    }
    if use_bias:
        shared["hb"] = np.asarray(head_b, np.float32).astype(NPBF16)[None, :]
        shared["b0"] = np.asarray(t0_b, np.float32).astype(NPBF16)[None, :]
        shared["b1"] = np.asarray(t1_b, np.float32).astype(NPBF16)[None, :]
        shared["pb0"] = np.asarray(t0_pb, np.float32).astype(NPBF16)[None, :]
        shared["pb1"] = np.asarray(t1_pb, np.float32).astype(NPBF16)[None, :]

    in_maps = []
    for c in range(NCORES):
        m = dict(shared)
        m["hidx"] = _wrap_idxs(head_tok[c], tpc)
        m["idx0"] = _wrap_idxs(c0_tok[c], cap0)
        m["idx1"] = _wrap_idxs(c1_tok[c], cap1)
        m["hsh"] = labshift(head_tok[c], head_labels, nbh, nchh, tpc)
        m["sh0"] = labshift(c0_tok[c], labels - CUTOFFS[0], nb0, nch0, c0_valid[c])
        m["sh1"] = labshift(c1_tok[c], labels - CUTOFFS[1], nb1, nch1, c1_valid[c])
        in_maps.append(m)
    return plan, in_maps


def assemble_loss(plan, outs):
    """outs: list of per-core [128, ncols] f32 arrays -> mean loss (f64)."""
    ntok = plan["ntok"]
    labels = plan["labels"]
    tpc = plan["tpc"]
    nbh = tpc // 128
    nb0 = plan["cap0"] // 128
    nb1 = plan["cap1"] // 128
    total = 0.0
    for c in range(NCORES):
        o = np.asarray(outs[c], np.float64)
        col = 0
        for tok_list, nb, valid in (
            (plan["head_tok"][c], nbh, tpc),
            (plan["c0_tok"][c], nb0, plan["c0_valid"][c]),
            (plan["c1_tok"][c], nb1, plan["c1_valid"][c]),
        ):
            se = o[:, col : col + nb].T.reshape(-1)[:valid]
            ll = o[:, col + nb : col + 2 * nb].T.reshape(-1)[:valid]
            w = (labels[tok_list[:valid]] != 0).astype(np.float64)
            total += float(np.dot(w, np.log(se) - ll))
            col += 2 * nb
    return total / ntok


_CACHE = {}


def kernel(inp, labels, head_W, head_b, t0_pW, t0_pb, t0_W, t0_b,
           t1_pW, t1_pb, t1_W, t1_b):
    plan, in_maps = make_plan_and_maps(
        inp, labels, head_W, head_b, t0_pW, t0_pb, t0_W, t0_b,
        t1_pW, t1_pb, t1_W, t1_b)
    key = (plan["ntok"], plan["tpc"], plan["cap0"], plan["cap1"], plan["use_bias"])
    if key not in _CACHE:
        _CACHE[key] = build_graph(plan)
    nc = _CACHE[key]
    res = run_bass_kernel_spmd(nc, in_maps, core_ids=list(range(NCORES)))
    outs = [res.results[c]["out"] for c in range(NCORES)]
    loss = assemble_loss(plan, outs)
    return np.asarray(loss, dtype=np.float32)


# revision 25
# speedup vs baseline: 33.9088x; 1.3534x over previous
"""Adaptive-softmax cross-entropy loss on 8 Trainium2 NeuronCores.

Strategy (token-parallel with label-sorted routing):
  * Tokens are sorted by label on the host. Each core takes a contiguous
    chunk of 512 sorted tokens for the head softmax (vocab 16002), plus a
    contiguous chunk of the cluster-0 / cluster-1 token runs for the two
    tail softmaxes (vocab 12000 / 8000, with low-rank input projections).
  * Each core gathers its token rows from HBM with a transposing
    dma_gather (bf16), so activations land directly in [K,128]-tile layout
    for the TensorEngine.
  * Logits never need a max-subtraction here (|logit| ~ 4), so streaming
    softmax is: matmul chunk -> Exp on the Scalar engine with fused
    accum_out (per-token sum of exps), plus one fused DVE
    scalar_tensor_tensor per chunk that extracts the label logit via an
    iota==label one-hot multiply-reduce.
  * Device outputs per-token (sum_exp, label_logit) pairs; host finishes
    with log() and the masked mean (tiny O(tokens) work).
"""

import math
from contextlib import ExitStack

import numpy as np
import ml_dtypes

import concourse.bass as bass
import concourse.mybir as mybir
import concourse.tile as tile
from concourse import bacc
from concourse.bass_utils import run_bass_kernel_spmd
from concourse.masks import make_identity

CUTOFFS = (16000, 28000, 36000)
HID = 1024
NCORES = 8
CH = 1024  # vocab chunk width (2 PSUM banks)
BF16 = mybir.dt.bfloat16
FP8 = mybir.dt.float8e4
F32 = mybir.dt.float32
NPBF16 = ml_dtypes.bfloat16
NPFP8 = ml_dtypes.float8_e4m3  # TRN FP8_EXP4: max +-240, matches exactly
PAD_LABEL = -100000.0
import os as _os
USE_FP8 = _os.environ.get("KERNEL_FP8", "1") == "1"  # fp8 DoubleRow matmuls

# ---------------------------------------------------------------------------
# Workaround for this container's walrus build: CoreV3 codegen accepts only
# ONE embedded sync-wait per instruction, while Tile emits instructions whose
# sync_info carries one wait per producing logical processor. Legalize after
# scheduling: hoist all-but-one wait onto same-engine NoOps inserted directly
# before the instruction (same-engine program order makes this equivalent).
_nop_counter = [0]


def _legalize_sync_waits(nc, max_waits=1):
    for fn in nc.m.functions:
        for blk in fn.blocks:
            insts = blk.instructions
            if not any(
                inst.sync_info is not None
                and inst.sync_info.on_wait
                and len(inst.sync_info.on_wait) > max_waits
                for inst in insts
            ):
                continue
            new = []
            for inst in insts:
                si = inst.sync_info
                waits = list(si.on_wait) if (si is not None and si.on_wait) else []
                if len(waits) > max_waits:
                    for w in waits[:-max_waits]:
                        _nop_counter[0] += 1
                        nop = mybir.InstNoOp(
                            name=f"LW-{_nop_counter[0]}", ins=[], outs=[]
                        )
                        nop.engine = inst.engine
                        nop.sync_info = mybir.SyncInfo(on_wait=[w], on_update=[])
                        nc.register_instruction(nop, overwrite=True)
                        new.append(nop)
                    inst.sync_info = mybir.SyncInfo(
                        on_wait=waits[-max_waits:],
                        on_update=list(si.on_update) if si.on_update else [],
                    )
                new.append(inst)
            blk.instructions = new
# ---------------------------------------------------------------------------


def _cdiv(a, b):
    return (a + b - 1) // b


def _wrap_idxs(idxs, num):
    """dma_gather index layout: idx i lives at [i % 16, i // 16], and the
    16-partition block is replicated to all 8 gpsimd cores (128 partitions)."""
    assert num % 16 == 0 and len(idxs) == num
    a = np.asarray(idxs, np.int16).reshape(num // 16, 16).T  # [16, num/16]
    return np.tile(a, (8, 1))  # [128, num/16]


def build_graph(plan, reps=1):
    """One SPMD graph, identical for all 8 cores.

    reps > 1 unrolls the whole kernel body back-to-back inside the NEFF so
    a timing harness can measure marginal (steady-state) per-rep cost,
    cancelling host/dispatch overhead: t = (T(K) - T(1)) / (K - 1)."""
    ntok = plan["ntok"]
    tpc = plan["tpc"]  # head tokens per core (multiple of 128)
    cap0, cap1 = plan["cap0"], plan["cap1"]  # tail token capacity per core
    nbh, nb0, nb1 = tpc // 128, cap0 // 128, cap1 // 128
    nchh, nch0, nch1 = _cdiv(16002, CH), _cdiv(12000, CH), _cdiv(8000, CH)
    use_bias = plan["use_bias"]
    ncols = plan["ncols"]

    fp8 = plan.get("fp8", False)
    WDT = FP8 if fp8 else BF16

    nc = bacc.Bacc(num_devices=NCORES)

    xt = nc.declare_dram_parameter("xt", [ntok, HID], BF16, isOutput=False)
    hw = nc.declare_dram_parameter("hw", [HID, 16002], WDT, isOutput=False)
    p0 = nc.declare_dram_parameter("p0", [HID, 256], WDT, isOutput=False)
    w0 = nc.declare_dram_parameter("w0", [256, 12000], WDT, isOutput=False)
    p1 = nc.declare_dram_parameter("p1", [HID, 64], BF16, isOutput=False)
    w1 = nc.declare_dram_parameter("w1", [64, 8000], BF16, isOutput=False)
    hidx = nc.declare_dram_parameter("hidx", [128, tpc // 16], mybir.dt.int16, isOutput=False)
    idx0 = nc.declare_dram_parameter("idx0", [128, cap0 // 16], mybir.dt.int16, isOutput=False)
    idx1 = nc.declare_dram_parameter("idx1", [128, cap1 // 16], mybir.dt.int16, isOutput=False)
    hsh = nc.declare_dram_parameter("hsh", [128, nbh, nchh], F32, isOutput=False)
    sh0 = nc.declare_dram_parameter("sh0", [128, nb0, nch0], F32, isOutput=False)
    sh1 = nc.declare_dram_parameter("sh1", [128, nb1, nch1], F32, isOutput=False)
    if use_bias:
        hb = nc.declare_dram_parameter("hb", [1, 16002], BF16, isOutput=False)
        b0 = nc.declare_dram_parameter("b0", [1, 12000], BF16, isOutput=False)
        b1 = nc.declare_dram_parameter("b1", [1, 8000], BF16, isOutput=False)
        pb0 = nc.declare_dram_parameter("pb0", [1, 256], BF16, isOutput=False)
        pb1 = nc.declare_dram_parameter("pb1", [1, 64], BF16, isOutput=False)
    out = nc.declare_dram_parameter("out", [128, ncols], F32, isOutput=True)

    Exp = mybir.ActivationFunctionType.Exp

    with tile.TileContext(nc) as tc:
        with ExitStack() as ctx:
            const = ctx.enter_context(tc.tile_pool(name="const", bufs=1))
            wpool = ctx.enter_context(tc.tile_pool(name="w", bufs=2))
            spool = ctx.enter_context(tc.tile_pool(name="scratch", bufs=3))

            # --- setup: indices / labels / constants (outside rep loop) ---
            hidx_sb = const.tile([128, tpc // 16], mybir.dt.int16)
            nc.sync.dma_start(out=hidx_sb[:, :], in_=hidx[:, :])
            idx0_sb = const.tile([128, cap0 // 16], mybir.dt.int16)
            nc.sync.dma_start(out=idx0_sb[:, :], in_=idx0[:, :])
            idx1_sb = const.tile([128, cap1 // 16], mybir.dt.int16)
            nc.sync.dma_start(out=idx1_sb[:, :], in_=idx1[:, :])

            hsh_sb = const.tile([128, nbh, nchh], F32)
            nc.sync.dma_start(out=hsh_sb[:, :, :], in_=hsh[:, :, :])
            sh0_sb = const.tile([128, nb0, nch0], F32)
            nc.sync.dma_start(out=sh0_sb[:, :, :], in_=sh0[:, :, :])
            sh1_sb = const.tile([128, nb1, nch1], F32)
            nc.sync.dma_start(out=sh1_sb[:, :, :], in_=sh1[:, :, :])

            iota_i = const.tile([128, CH], mybir.dt.int32)
            nc.gpsimd.iota(iota_i[:, :], pattern=[[1, CH]], base=0, channel_multiplier=0)
            iota_f = const.tile([128, CH], F32)
            nc.vector.tensor_copy(iota_f[:, :], iota_i[:, :])

            identity = const.tile([128, 128], BF16)
            make_identity(nc, identity[:, :])

            ones1 = const.tile([1, 128], BF16)
            nc.vector.memset(ones1[:, :], 1.0)

            bias_sb = {}
            if use_bias:
                for name, ap, n in (
                    ("hb", hb, 16002), ("b0", b0, 12000), ("b1", b1, 8000),
                    ("pb0", pb0, 256), ("pb1", pb1, 64),
                ):
                    t = const.tile([1, n], BF16, tag=f"bias_{name}")
                    nc.sync.dma_start(out=t[:, :], in_=ap[:, :])
                    bias_sb[name] = t

            p0_sb = const.tile([128, 8, 256], WDT)
            nc.sync.dma_start(out=p0_sb[:, :, :], in_=p0.ap().rearrange("(c p) n -> p c n", p=128))
            p1_sb = const.tile([128, 8, 64], BF16)
            nc.sync.dma_start(out=p1_sb[:, :, :], in_=p1.ap().rearrange("(c p) n -> p c n", p=128))

            # persistent result tiles (rewritten each rep)
            xh = const.tile([128, 8, tpc], BF16)
            x0 = const.tile([128, 8, cap0], BF16)
            x1 = const.tile([128, 8, cap1], BF16)
            if fp8:
                xh8 = const.tile([128, 8, tpc], FP8)
                x08 = const.tile([128, 8, cap0], FP8)
            hT0 = const.tile([128, 2, cap0], WDT)
            hT1 = const.tile([64, 1, cap1], BF16)
            hse = const.tile([128, nbh, nchh], F32)
            hll = const.tile([128, nbh, nchh], F32)
            se0 = const.tile([128, nb0, nch0], F32)
            ll0 = const.tile([128, nb0, nch0], F32)
            se1 = const.tile([128, nb1, nch1], F32)
            ll1 = const.tile([128, nb1, nch1], F32)
            out_sb = const.tile([128, ncols], F32)

            def emit_body():
                # gather this core's token rows (transposed, bf16)
                nc.gpsimd.dma_gather(
                    xh[:, :, :], xt[:, :], hidx_sb[:, :],
                    num_idxs=tpc, num_idxs_reg=tpc, elem_size=HID, transpose=True,
                )
                nc.gpsimd.dma_gather(
                    x0[:, :, :], xt[:, :], idx0_sb[:, :],
                    num_idxs=cap0, num_idxs_reg=cap0, elem_size=HID, transpose=True,
                )
                nc.gpsimd.dma_gather(
                    x1[:, :, :], xt[:, :], idx1_sb[:, :],
                    num_idxs=cap1, num_idxs_reg=cap1, elem_size=HID, transpose=True,
                )
                if fp8:
                    nc.vector.tensor_copy(xh8[:, :, :], xh[:, :, :])
                    nc.vector.tensor_copy(x08[:, :, :], x0[:, :, :])
                    xhm, x0m = xh8, x08
                else:
                    xhm, x0m = xh, x0

                # --- tail projections: h = x @ pW (+pb) -> transpose [proj, tok] ---
                with tc.tile_pool(name="ppsum", bufs=2, space="PSUM") as ppsum:
                    for tb in range(nb0):
                        ph = ppsum.tile([128, 256], F32, tag="proj")
                        if fp8:
                            for c2 in range(4):
                                nc.tensor.matmul(
                                    ph[:, :],
                                    x0m[:, 2 * c2 : 2 * c2 + 2, bass.ts(tb, 128)],
                                    p0_sb[:, 2 * c2 : 2 * c2 + 2, :],
                                    start=(c2 == 0), stop=(c2 == 3 and not use_bias),
                                    perf_mode=mybir.MatmulPerfMode.DoubleRow,
                                )
                        else:
                            for c in range(8):
                                nc.tensor.matmul(
                                    ph[:, :], x0m[:, c, bass.ts(tb, 128)], p0_sb[:, c, :],
                                    start=(c == 0), stop=(c == 7 and not use_bias),
                                )
                        if use_bias:
                            nc.tensor.matmul(
                                ph[:, :], ones1[0:1, :], bias_sb["pb0"][0:1, :],
                                start=False, stop=True,
                            )
                        hsb = spool.tile([128, 256], BF16, tag="hsb")
                        nc.scalar.copy(hsb[:, :], ph[:, :])
                        for j in range(2):
                            pt = ppsum.tile([128, 128], BF16, tag="ptr")
                            nc.tensor.transpose(pt[:, :], hsb[:, bass.ts(j, 128)], identity[:, :])
                            nc.scalar.copy(hT0[:, j, bass.ts(tb, 128)], pt[:, :])
                    for tb in range(nb1):
                        ph = ppsum.tile([128, 64], F32, tag="proj")
                        for c in range(8):
                            nc.tensor.matmul(
                                ph[:, :], x1[:, c, bass.ts(tb, 128)], p1_sb[:, c, :],
                                start=(c == 0), stop=(c == 7 and not use_bias),
                            )
                        if use_bias:
                            nc.tensor.matmul(
                                ph[:, :], ones1[0:1, :], bias_sb["pb1"][0:1, :],
                                start=False, stop=True,
                            )
                        hsb = spool.tile([128, 64], BF16, tag="hsb")
                        nc.scalar.copy(hsb[:, :], ph[:, :])
                        pt = ppsum.tile([64, 128], BF16, tag="ptr")
                        nc.tensor.transpose(pt[:, :], hsb[:, :], identity[:, :])
                        nc.scalar.copy(hT1[:, 0, bass.ts(tb, 128)], pt[:, :])

                # --- streaming softmax over vocab chunks ---
                def stream(xT, KT, kpart, W_ap, V, nb, labsh, se_sb, ll_sb, bias,
                           spans, dr=False):
                    nch = _cdiv(V, CH)
                    Wr = W_ap.ap().rearrange("(c p) n -> p c n", p=kpart)
                    wdt = FP8 if dr else BF16
                    for ch in range(nch):
                        v0 = ch * CH
                        vw = min(CH, V - v0)
                        wt = wpool.tile([kpart, KT, vw], wdt, tag="wt")
                        nc.sync.dma_start(out=wt[:, :, :], in_=Wr[:, :, v0 : v0 + vw])
                        for tb in range(nb):
                            ps = lpsum.tile([128, CH], F32, tag="logits")
                            if dr:
                                for c2 in range(KT // 2):
                                    for s0 in range(0, vw, 512):
                                        sw = min(512, vw - s0)
                                        nc.tensor.matmul(
                                            ps[:, s0 : s0 + sw],
                                            xT[:, 2 * c2 : 2 * c2 + 2, bass.ts(tb, 128)],
                                            wt[:, 2 * c2 : 2 * c2 + 2, s0 : s0 + sw],
                                            start=(c2 == 0),
                                            stop=(c2 == KT // 2 - 1 and bias is None),
                                            perf_mode=mybir.MatmulPerfMode.DoubleRow,
                                        )
                            else:
                                for c in range(KT):
                                    for s0 in range(0, vw, 512):
                                        sw = min(512, vw - s0)
                                        nc.tensor.matmul(
                                            ps[:, s0 : s0 + sw],
                                            xT[:, c, bass.ts(tb, 128)],
                                            wt[:, c, s0 : s0 + sw],
                                            start=(c == 0),
                                            stop=(c == KT - 1 and bias is None),
                                        )
                            if bias is not None:
                                for s0 in range(0, vw, 512):
                                    sw = min(512, vw - s0)
                                    nc.tensor.matmul(
                                        ps[:, s0 : s0 + sw],
                                        ones1[0:1, :],
                                        bias[0:1, v0 + s0 : v0 + s0 + sw],
                                        start=False, stop=True,
                                    )
                            ex = spool.tile([128, CH], F32, tag="ex")
                            nc.scalar.activation(
                                ex[:, :vw], ps[:, :vw], Exp,
                                accum_out=se_sb[:, tb, ch : ch + 1],
                            )
                            if spans[tb][0] <= ch <= spans[tb][1]:
                                st = spool.tile([128, CH], F32, tag="st")
                                nc.vector.scalar_tensor_tensor(
                                    out=st[:, :vw],
                                    in0=iota_f[:, :vw],
                                    scalar=labsh[:, tb, ch : ch + 1],
                                    in1=ps[:, :vw],
                                    op0=mybir.AluOpType.is_equal,
                                    op1=mybir.AluOpType.mult,
                                    accum_out=ll_sb[:, tb, ch : ch + 1],
                                )

                with tc.tile_pool(name="lpsum", bufs=4, space="PSUM") as lpsum:
                    stream(xhm, 8, 128, hw, 16002, nbh, hsh_sb, hse, hll,
                           bias_sb.get("hb"), plan["hspans"], dr=fp8)
                    stream(hT0, 2, 128, w0, 12000, nb0, sh0_sb, se0, ll0,
                           bias_sb.get("b0"), plan["spans0"], dr=fp8)
                    stream(hT1, 1, 64, w1, 8000, nb1, sh1_sb, se1, ll1,
                           bias_sb.get("b1"), plan["spans1"])

                    # --- reduce per-chunk partials into output columns ---
                    col = 0
                    for se_sb, ll_sb, nb, nch, spans in (
                        (hse, hll, nbh, nchh, plan["hspans"]),
                        (se0, ll0, nb0, nch0, plan["spans0"]),
                        (se1, ll1, nb1, nch1, plan["spans1"]),
                    ):
                        for tb in range(nb):
                            lo, hi = spans[tb]
                            nc.vector.reduce_sum(
                                out=out_sb[:, col : col + 1],
                                in_=se_sb[:, tb, :], axis=mybir.AxisListType.X,
                            )
                            nc.vector.reduce_sum(
                                out=out_sb[:, col + nb : col + nb + 1],
                                in_=ll_sb[:, tb, lo : hi + 1],
                                axis=mybir.AxisListType.X,
                            )
                            col += 1
                        col += nb

                    nc.sync.dma_start(out=out[:, :], in_=out_sb[:, :])

            for _rep in range(reps):
                emit_body()

    nc.compile()
    _legalize_sync_waits(nc)
    return nc


def make_plan_and_maps(inp, labels, head_W, head_b, t0_pW, t0_pb, t0_W, t0_b,
                       t1_pW, t1_pb, t1_W, t1_b):
    X = np.ascontiguousarray(np.asarray(inp, np.float32).reshape(-1, HID))
    labels = np.asarray(labels).astype(np.int64).reshape(-1)
    ntok = X.shape[0]
    assert ntok % (NCORES * 128) == 0, ntok

    order = np.argsort(labels, kind="stable")
    slab = labels[order]
    head_labels = labels.copy()
    m0 = (labels >= CUTOFFS[0]) & (labels < CUTOFFS[1])
    m1 = (labels >= CUTOFFS[1]) & (labels < CUTOFFS[2])
    head_labels[m0] = CUTOFFS[0]
    head_labels[m1] = CUTOFFS[0] + 1

    tpc = ntok // NCORES
    # Round-robin deal of sorted tokens: core c slot s <- order[s*8 + c].
    # Every core's block b then spans the same global sorted-position range
    # [b*1024, (b+1)*1024), so the set of vocab chunks a block's labels can
    # hit is identical across cores -> static SPMD-uniform extraction sets.
    head_tok = [order[c::NCORES] for c in range(NCORES)]

    def block_spans(sorted_labels, nblocks, nch):
        """Per block: [lo_chunk, hi_chunk] of label//CH over its global range."""
        spans = []
        n = len(sorted_labels)
        for b in range(nblocks):
            g0, g1 = b * 128 * NCORES, min((b + 1) * 128 * NCORES, n)
            if g0 >= g1:
                spans.append((0, 0))
                continue
            lo = int(sorted_labels[g0]) // CH
            hi = int(sorted_labels[g1 - 1]) // CH
            spans.append((max(lo, 0), min(hi, nch - 1)))
        return tuple(spans)

    def split_cluster(lo, hi):
        toks = order[(slab >= lo) & (slab < hi)]  # sorted by label
        n = len(toks)
        per = _cdiv(max(n, 1), NCORES)
        cap = max(_cdiv(per, 128) * 128, 128)
        chunks, valid = [], []
        for c in range(NCORES):
            chunk = toks[c::NCORES]
            v = len(chunk)
            pad = np.zeros(cap - v, np.int64)
            chunks.append(np.concatenate([chunk, pad]))
            valid.append(v)
        return chunks, valid, cap, (slab[(slab >= lo) & (slab < hi)] - lo)

    c0_tok, c0_valid, cap0, c0_slab = split_cluster(CUTOFFS[0], CUTOFFS[1])
    c1_tok, c1_valid, cap1, c1_slab = split_cluster(CUTOFFS[1], CUTOFFS[2])

    nbh, nb0, nb1 = tpc // 128, cap0 // 128, cap1 // 128
    nchh, nch0, nch1 = _cdiv(16002, CH), _cdiv(12000, CH), _cdiv(8000, CH)
    ncols = 2 * (nbh + nb0 + nb1)

    use_bias = any(
        float(np.abs(np.asarray(b, np.float32)).max()) > 0
        for b in (head_b, t0_b, t1_b, t0_pb, t1_pb)
    )

    hspans = block_spans(head_labels[order], nbh, nchh)
    spans0 = block_spans(c0_slab, nb0, nch0)
    spans1 = block_spans(c1_slab, nb1, nch1)

    plan = dict(ntok=ntok, tpc=tpc, cap0=cap0, cap1=cap1, ncols=ncols,
                use_bias=use_bias, head_tok=head_tok, c0_tok=c0_tok,
                c1_tok=c1_tok, c0_valid=c0_valid, c1_valid=c1_valid,
                labels=labels, head_labels=head_labels, fp8=USE_FP8,
                hspans=hspans, spans0=spans0, spans1=spans1)

    def labshift(tok_list, lab_vals, nb, nch, valid):
        """[128, nb, nch] f32: label - chunk_base, PAD_LABEL on padding."""
        a = np.full((nb * 128,), PAD_LABEL, np.float32)
        a[:valid] = lab_vals[tok_list[:valid]].astype(np.float32)
        a = a.reshape(nb, 128).T  # [128, nb]
        base = (np.arange(nch, dtype=np.float32) * CH)[None, None, :]
        return np.ascontiguousarray(a[:, :, None] - base)

    Xb = X.astype(NPBF16)
    wdt = NPFP8 if USE_FP8 else NPBF16
    shared = {
        "xt": Xb,
        "hw": np.asarray(head_W, np.float32).astype(wdt),
        "p0": np.asarray(t0_pW, np.float32).astype(wdt),
        "w0": np.asarray(t0_W, np.float32).astype(wdt),
        "p1": np.asarray(t1_pW, np.float32).astype(NPBF16),
        "w1": np.asarray(t1_W, np.float32).astype(NPBF16),
    }
    if use_bias:
        shared["hb"] = np.asarray(head_b, np.float32).astype(NPBF16)[None, :]
        shared["b0"] = np.asarray(t0_b, np.float32).astype(NPBF16)[None, :]
        shared["b1"] = np.asarray(t1_b, np.float32).astype(NPBF16)[None, :]
        shared["pb0"] = np.asarray(t0_pb, np.float32).astype(NPBF16)[None, :]
        shared["pb1"] = np.asarray(t1_pb, np.float32).astype(NPBF16)[None, :]

    in_maps = []
    for c in range(NCORES):
        m = dict(shared)
        m["hidx"] = _wrap_idxs(head_tok[c], tpc)
        m["idx0"] = _wrap_idxs(c0_tok[c], cap0)
        m["idx1"] = _wrap_idxs(c1_tok[c], cap1)
        m["hsh"] = labshift(head_tok[c], head_labels, nbh, nchh, tpc)
        m["sh0"] = labshift(c0_tok[c], labels - CUTOFFS[0], nb0, nch0, c0_valid[c])
        m["sh1"] = labshift(c1_tok[c], labels - CUTOFFS[1], nb1, nch1, c1_valid[c])
        in_maps.append(m)
    return plan, in_maps


def assemble_loss(plan, outs):
    """outs: list of per-core [128, ncols] f32 arrays -> mean loss (f64)."""
    ntok = plan["ntok"]
    labels = plan["labels"]
    tpc = plan["tpc"]
    nbh = tpc // 128
    nb0 = plan["cap0"] // 128
    nb1 = plan["cap1"] // 128
    total = 0.0
    for c in range(NCORES):
        o = np.asarray(outs[c], np.float64)
        col = 0
        for tok_list, nb, valid in (
            (plan["head_tok"][c], nbh, tpc),
            (plan["c0_tok"][c], nb0, plan["c0_valid"][c]),
            (plan["c1_tok"][c], nb1, plan["c1_valid"][c]),
        ):
            se = o[:, col : col + nb].T.reshape(-1)[:valid]
            ll = o[:, col + nb : col + 2 * nb].T.reshape(-1)[:valid]
            w = (labels[tok_list[:valid]] != 0).astype(np.float64)
            total += float(np.dot(w, np.log(se) - ll))
            col += 2 * nb
    return total / ntok


_CACHE = {}


def kernel(inp, labels, head_W, head_b, t0_pW, t0_pb, t0_W, t0_b,
           t1_pW, t1_pb, t1_W, t1_b):
    plan, in_maps = make_plan_and_maps(
        inp, labels, head_W, head_b, t0_pW, t0_pb, t0_W, t0_b,
        t1_pW, t1_pb, t1_W, t1_b)
    key = (plan["ntok"], plan["tpc"], plan["cap0"], plan["cap1"],
           plan["use_bias"], plan["fp8"],
           plan["hspans"], plan["spans0"], plan["spans1"])
    if key not in _CACHE:
        _CACHE[key] = build_graph(plan)
    nc = _CACHE[key]
    res = run_bass_kernel_spmd(nc, in_maps, core_ids=list(range(NCORES)))
    outs = [res.results[c]["out"] for c in range(NCORES)]
    loss = assemble_loss(plan, outs)
    return np.asarray(loss, dtype=np.float32)
